# revision 25
# baseline (speedup 1.0000x reference)
"""Trainium2 Bass kernel for nn_DecoderBlock (B=8, S=1024, D=256, H=4 heads
of full width 256, FF=1024).

Strategy: pure data parallelism — B=8 batch elements across 8 NeuronCores,
zero collectives. Per core, one full decoder block in "transposed" activation
layout (features on SBUF partitions, tokens on the free dim).

v5: all large matmuls run fp8(e4m3) DoubleRow (2 contraction rows/cycle);
host-folded attention projections kill PSUM-evacuation traffic (only
ACT/DVE can read PSUM):

  scores_h = x^T M_h x   with M_h = wk_h wq_h^T   (no K projection)
  mh       = sum_h G_h^T (x^T exp_h / Z_h),  G_h = wv_h wo_h  (no V proj;
             the normalize rides the x^T exp psum evacuation)

Z comes from a ones(1/16) matmul so recip lands pre-scaled for fp8 u/Z.
Weight scales (x16/x64) fold into the exp scale and 1024x/256x-scaled
LN residuals (LN is scale-invariant; eps rescales). LN1 gamma/beta fold
into ff1 weights/bias and the r2 residual op.

Schedule notes (from trace analysis):
 - The PE HAM clock-gate halves the clock when the PE idles; every
   potential bubble is padded with warm matmuls into a dedicated psum
   bank (WAW-only, so a warm never waits on another engine).
 - The ACT function-table reload costs ~1.3us; any Square/Sqrt between
   attention Exps forces a reload pair. The two in-attention LN1 chains
   are emitted back-to-back as ONE episode; all other non-Exp ACT work
   (relu, LN2 apply) is deferred until after the last Exp.
 - Inputs arrive as 9 consolidated DMAs split across the sync + scalar
   HWDGE queues.

The attention_mask input is all ones per the problem spec (causal mask
only); if a mask with zeros shows up, we fall back to a numpy reference.
"""

import numpy as np
import ml_dtypes

import concourse.bass as bass
import concourse.mybir as mybir
import concourse.tile as tile
from concourse import bacc
from concourse.bass_utils import run_bass_kernel_spmd

F32 = mybir.dt.float32
BF16 = mybir.dt.bfloat16
F8 = mybir.dt.float8e4
AF = mybir.ActivationFunctionType
ALU = mybir.AluOpType
DR = mybir.MatmulPerfMode.DoubleRow

N_CORES = 8
B, S, D, H, E, HE, FF = 8, 1024, 256, 4, 256, 1024, 1024
SC = 512          # attention query chunk
NJ = S // SC
ND = D // 128
NF = FF // 128
NT = S // 128
LN_EPS = 1e-5
WS = 16.0
MS = 64.0                            # M/G host-fold scale
SCALE8 = (1.0 / 16.0) / MS           # exp reads 64x-scaled scores
EPS1 = (MS * 16.0) ** 2 * LN_EPS     # LN1 eps for 1024x-scaled r1
EPS2 = 65536.0 * LN_EPS              # LN2 eps for 256x-scaled r2
CW = 256
POST = [(0, 256), (256, 512), (512, 768), (768, 1024)]

_CACHE = {}


def _build():
    nc = bacc.Bacc("TRN2", target_bir_lowering=False, debug=False,
                   num_devices=N_CORES)

    # ---- DRAM parameters (consolidated: 9 input DMAs total) ----
    x8_d = nc.dram_tensor("x8", [128, ND, S], F8, kind="ExternalInput")
    xS8_d = nc.dram_tensor("xS8", [128, 4, 2, D], F8, kind="ExternalInput")
    x1k_d = nc.dram_tensor("x1k", [ND, 128, S], BF16, kind="ExternalInput")
    M8_d = nc.dram_tensor("M8", [128, H, ND, D], F8, kind="ExternalInput")
    G8_d = nc.dram_tensor("G8", [128, H, ND, D], F8, kind="ExternalInput")
    ff18_d = nc.dram_tensor("ff18", [128, ND, FF], F8, kind="ExternalInput")
    ff28_d = nc.dram_tensor("ff28", [128, 4, 2, D], F8, kind="ExternalInput")
    # packed per-partition params: wo_b1k(2) ff1b16(8) rb2(2) g256(2)
    # ln2_g(2) ln2_b(2)
    par_d = nc.dram_tensor("par", [128, 18], F32, kind="ExternalInput")
    out_d = nc.dram_tensor("out", [ND, 128, S], F32, kind="ExternalOutput")

    with tile.TileContext(nc) as tc:
        with tc.tile_pool(name="consts", bufs=1) as consts, \
             tc.tile_pool(name="acts", bufs=1) as acts, \
             tc.tile_pool(name="work", bufs=2) as work, \
             tc.tile_pool(name="psMM", bufs=2, space="PSUM") as psMM, \
             tc.tile_pool(name="psOP", bufs=3, space="PSUM") as psOP, \
             tc.tile_pool(name="psC", bufs=2, space="PSUM") as psC, \
             tc.tile_pool(name="psW", bufs=1, space="PSUM") as psW:

            ones_bf = consts.tile([128, 128], BF16, tag="onesb", name="onesb")
            nc.vector.memset(ones_bf[:], 1.0)
            # Z-reduction operand: 1/16 so zb = recip(Z/16) = 16/Z lands
            # pre-scaled for the fp8 range of u/Z
            ones8 = consts.tile([128, 2, 128], F8, tag="ones8", name="ones8")
            nc.gpsimd.memset(ones8[:], 1.0 / 16.0)
            invd_bf = consts.tile([128, 128], BF16, tag="invd", name="invd")
            nc.vector.memset(invd_bf[:], 1.0 / D)
            eps1_t = consts.tile([128, 1], F32, tag="eps1", name="eps1")
            nc.vector.memset(eps1_t[:], EPS1)
            eps2_t = consts.tile([128, 1], F32, tag="eps2", name="eps2")
            nc.vector.memset(eps2_t[:], EPS2)
            # multiplicative causal mask for a diagonal 128x128 block of a
            # transposed [key, query] exp tile: 1 where query >= key else 0
            cmaskf = consts.tile([128, 128], F32, tag="cmaskf", name="cmaskf")
            nc.gpsimd.memset(cmaskf[:], 1.0)
            nc.gpsimd.affine_select(
                out=cmaskf[:], in_=cmaskf[:],
                compare_op=ALU.is_ge, fill=0.0,
                base=0, pattern=[[1, 128]], channel_multiplier=-1,
            )
            cmask8 = consts.tile([128, 128], F8, tag="cmask", name="cmask")
            nc.vector.tensor_copy(out=cmask8[:], in_=cmaskf[:])

            # warm matmuls write a dedicated psum bank (WAW on itself only,
            # so a warm NEVER waits on another engine): keep the PE HAM
            # clock-gate at 8/8 by padding every potential PE bubble
            warm_rhs = consts.tile([128, SC], BF16, tag="wrhs", name="wrhs")
            nc.gpsimd.memset(warm_rhs[:], 0.0)
            wp = psW.tile([128, SC], F32, tag="w", name="wp")

            def warm_pe(n, wide=True):
                for _ in range(n):
                    if wide:
                        nc.tensor.matmul(wp[:], ones_bf[:], warm_rhs[:],
                                         start=True, stop=True,
                                         skip_group_check=True)
                    else:
                        nc.tensor.matmul(wp[:, 0:128], ones_bf[:],
                                         warm_rhs[:, 0:128],
                                         start=True, stop=True,
                                         skip_group_check=True)

            warm_pe(12)

            # ---- inputs: consolidated DMAs split across the two HWDGE
            # queues in first-use order
            def loadq(eng, dram, shape, dt, tag):
                t = consts.tile(shape, dt, tag=tag, name=tag)
                eng.dma_start(out=t[:], in_=dram[:])
                return t

            x8 = loadq(nc.sync, x8_d, [128, ND, S], F8, "x8")
            M8t = loadq(nc.sync, M8_d, [128, H, ND, D], F8, "M8")
            xS8t = loadq(nc.sync, xS8_d, [128, 4, 2, D], F8, "xS8")
            G8t = loadq(nc.scalar, G8_d, [128, H, ND, D], F8, "G8")
            par = loadq(nc.sync, par_d, [128, 18], F32, "par")
            x1k = []
            for i in range(ND):
                t = consts.tile([128, S], BF16, tag=f"x1k{i}", name=f"x1k{i}")
                nc.sync.dma_start(out=t[:], in_=x1k_d[i])
                x1k.append(t)
            ff18 = loadq(nc.scalar, ff18_d, [128, ND, FF], F8, "ff18")
            ff28t = loadq(nc.scalar, ff28_d, [128, 4, 2, D], F8, "ff28")

            wo_b1k = [par[:, 0:1], par[:, 1:2]]
            ff1b16 = [par[:, 2 + i:3 + i] for i in range(NF)]
            rb2 = [par[:, 10:11], par[:, 11:12]]
            g256 = [par[:, 12:13], par[:, 13:14]]
            ln2_g = [par[:, 14:15], par[:, 15:16]]
            ln2_b = [par[:, 16:17], par[:, 17:18]]

            # ---- persistent activation tiles ----
            qm8 = [acts.tile([128, 2, S], F8, tag=f"qm{h}", name=f"qm{h}")
                   for h in range(H)]
            un8 = [acts.tile([128, 2, S], F8, tag=f"un{h}", name=f"un{h}")
                   for h in range(H)]
            x18 = acts.tile([128, 2, S], F8, tag="x18", name="x18")
            x1z = [acts.tile([128, S], BF16, tag=f"x1z{d0}", name=f"x1z{d0}")
                   for d0 in range(ND)]
            h8 = [acts.tile([128, 2, S], F8, tag=f"h8{c}", name=f"h8{c}")
                  for c in range(4)]
            r1b = [acts.tile([128, S], BF16, tag=f"r1b{d0}", name=f"r1b{d0}")
                   for d0 in range(ND)]
            r2b = [acts.tile([128, S], BF16, tag=f"r2b{d0}", name=f"r2b{d0}")
                   for d0 in range(ND)]

            # ---- work-item pump ----
            prefq = []
            postq = []

            def pump(npre, npost):
                for _ in range(npre):
                    if prefq:
                        prefq.pop(0)()
                for _ in range(npost):
                    if postq:
                        postq.pop(0)()

            def proj_qm(h, e0, j):
                # qmT[e0 block, j cols] = (M_h^T x)[.,.]  (scale 64);
                # evacuation engine: ACT Copy is table-free, so j0 segs
                # (ACT has exp slack there) take ACT, j1 takes DVE
                cols = slice(j * SC, (j + 1) * SC)
                p = psMM.tile([128, SC], F32, tag="mm", name="mm")
                c0 = e0 * 128
                nc.tensor.matmul(p[:], M8t[:, h, :, c0:c0 + 128],
                                 x8[:, :, cols], start=True, stop=True,
                                 perf_mode=DR)
                if j == 0:
                    nc.scalar.copy(out=qm8[h][:, e0, cols], in_=p[:])
                else:
                    nc.vector.tensor_copy(out=qm8[h][:, e0, cols], in_=p[:])

            def seg_prefetch(h, j):
                return [lambda: proj_qm(h, 0, j), lambda: proj_qm(h, 1, j)]

            # ---- post-attention pipeline, per 256-token chunk ----
            def post_chunk_parts(a, b, tail):
                cw = b - a
                cols = slice(a, b)
                cell = {}
                # tail chunks run after attention: spread their psums over
                # the then-idle attention pools to decouple pipeline stages
                wo_pool, wo_tag = (psMM, "mm") if tail else (psC, "mmc")
                f1_pool, f1_tag = (psOP, "op") if tail else (psC, "mmc")

                def wo_item(d0):
                    pp = wo_pool.tile([128, SC], F32, tag=wo_tag, name="mmc")
                    for c in range(H):
                        nc.tensor.matmul(
                            pp[:, 0:cw], G8t[:, c, :, d0 * 128:(d0 + 1) * 128],
                            un8[c][:, :, cols], start=(c == 0),
                            stop=(c == H - 1), perf_mode=DR)
                    # r1b = 1024*(mh + wo_b + x), bf16 (LN scale-invariant)
                    nc.vector.scalar_tensor_tensor(
                        out=r1b[d0][:, cols], in0=pp[:, 0:cw],
                        scalar=wo_b1k[d0], in1=x1k[d0][:, cols],
                        op0=ALU.add, op1=ALU.add)

                def stats_item(src, key):
                    # mup in [:, 0:cw], m2p in [:, CW:CW+cw] of ONE psum
                    # bank: the first start=True matmul marks the whole 2KB
                    # bank pending-zero, later start=False write through.
                    st = wo_pool.tile([128, SC], F32, tag=wo_tag, name="st")
                    sq = [work.tile([128, CW], BF16, tag=f"sq{d0}",
                                    name=f"sq{d0}") for d0 in range(ND)]
                    for d0 in range(ND):
                        nc.gpsimd.tensor_mul(out=sq[d0][:, 0:cw],
                                             in0=src[d0][:, cols],
                                             in1=src[d0][:, cols])
                    for d0 in range(ND):
                        nc.tensor.matmul(st[:, 0:cw], invd_bf[:],
                                         src[d0][:, cols], start=(d0 == 0),
                                         stop=False, skip_group_check=True)
                    for d0 in range(ND):
                        nc.tensor.matmul(st[:, CW:CW + cw], invd_bf[:],
                                         sq[d0][:, 0:cw], start=False,
                                         stop=(d0 == ND - 1),
                                         skip_group_check=True)
                    cell[key] = st

                def chain_common(st, eps_t):
                    mup = st[:, 0:cw]
                    m2p = st[:, CW:CW + cw]
                    musq = work.tile([128, CW], F32, tag="musq", name="musq")
                    nc.scalar.activation(out=musq[:, 0:cw], in_=mup,
                                         func=AF.Square)
                    var = work.tile([128, CW], F32, tag="var", name="var")
                    nc.vector.tensor_sub(out=var[:, 0:cw], in0=m2p,
                                         in1=musq[:, 0:cw])
                    sd = work.tile([128, CW], F32, tag="sd", name="sd")
                    nc.scalar.activation(out=sd[:, 0:cw], in_=var[:, 0:cw],
                                         func=AF.Sqrt, bias=eps_t)
                    rstd = work.tile([128, CW], F32, tag="rstd", name="rstd")
                    nc.vector.reciprocal_approx_fast(out=rstd[:, 0:cw],
                                                     in_=sd[:, 0:cw])
                    return mup, rstd

                def chain1_item():
                    mup, rstd = chain_common(cell["st1"], eps1_t)
                    for d0 in range(ND):
                        u = work.tile([128, CW], F32, tag=f"u{d0}",
                                      name=f"u{d0}")
                        nc.vector.tensor_sub(out=u[:, 0:cw],
                                             in0=r1b[d0][:, cols], in1=mup)
                        # z (normalized, un-affine) feeds ff1 (gamma/beta
                        # folded into ff1 weights); x1z = 256*g1*z for the
                        # r2 residual (Pool does plain tensor-tensor only,
                        # so pre-scale rstd by gamma on DVE)
                        nc.vector.tensor_mul(out=x18[:, d0, cols],
                                             in0=u[:, 0:cw], in1=rstd[:, 0:cw])
                        gr = work.tile([128, CW], F32, tag=f"gr{d0}",
                                       name=f"gr{d0}")
                        nc.vector.tensor_scalar(
                            out=gr[:, 0:cw], in0=rstd[:, 0:cw],
                            scalar1=g256[d0], scalar2=None, op0=ALU.mult)
                        nc.gpsimd.tensor_mul(out=x1z[d0][:, cols],
                                             in0=u[:, 0:cw], in1=gr[:, 0:cw])

                def ff1_item(f0):
                    fp = f1_pool.tile([128, SC], F32, tag=f1_tag, name="mmc")
                    c0 = f0 * 128
                    nc.tensor.matmul(fp[:, 0:cw], ff18[:, :, c0:c0 + 128],
                                     x18[:, :, cols], start=True, stop=True,
                                     perf_mode=DR)
                    if tail and f0 % 2 == 0:
                        # ACT is exp-free after attention
                        nc.scalar.activation(
                            out=h8[f0 // 2][:, f0 % 2, cols], in_=fp[:, 0:cw],
                            func=AF.Relu, bias=ff1b16[f0])
                    else:
                        nc.vector.tensor_scalar(
                            out=h8[f0 // 2][:, f0 % 2, cols], in0=fp[:, 0:cw],
                            scalar1=ff1b16[f0], scalar2=0.0,
                            op0=ALU.add, op1=ALU.max)

                def ff2_item(d0):
                    fp = psC.tile([128, SC], F32, tag="mmc", name="mmc")
                    for c in range(4):
                        nc.tensor.matmul(
                            fp[:, 0:cw],
                            ff28t[:, c, :, d0 * 128:(d0 + 1) * 128],
                            h8[c][:, :, cols], start=(c == 0), stop=(c == 3),
                            perf_mode=DR)
                    # r2b = 256*(ff + ff2_b + x1): x1z carries 256*g1*z,
                    # rb2 carries 256*(ff2_b + b1)
                    nc.vector.scalar_tensor_tensor(
                        out=r2b[d0][:, cols], in0=fp[:, 0:cw],
                        scalar=rb2[d0], in1=x1z[d0][:, cols],
                        op0=ALU.add, op1=ALU.add)

                def chain2_item():
                    mup, rstd = chain_common(cell["st2"], eps2_t)
                    for d0 in range(ND):
                        u = work.tile([128, CW], F32, tag=f"u{d0}",
                                      name=f"u{d0}")
                        nc.vector.tensor_sub(out=u[:, 0:cw],
                                             in0=r2b[d0][:, cols], in1=mup)
                        t = work.tile([128, CW], F32, tag=f"t{d0}",
                                      name=f"t{d0}")
                        nc.vector.tensor_mul(out=t[:, 0:cw], in0=u[:, 0:cw],
                                             in1=rstd[:, 0:cw])
                        ow = work.tile([128, CW], F32, tag=f"ow{d0}",
                                       name=f"ow{d0}")
                        nc.scalar.activation(out=ow[:, 0:cw], in_=t[:, 0:cw],
                                             func=AF.Identity,
                                             scale=ln2_g[d0], bias=ln2_b[d0])
                        nc.sync.dma_start(out=out_d[d0][:, cols],
                                          in_=ow[:, 0:cw])

                return dict(
                    pre=[lambda: wo_item(0), lambda: wo_item(1),
                         lambda: stats_item(r1b, "st1")],
                    chain1=chain1_item,
                    mid=[*(lambda f0=f0: ff1_item(f0) for f0 in range(NF)),
                         lambda: ff2_item(0), lambda: ff2_item(1),
                         lambda: stats_item(r2b, "st2")],
                    chain2=chain2_item,
                )

            # ---- attention segment: head h, query window [a, b) ----
            def seg_attention(h, a, b, pre_rate, post_rate):
                cw = b - a
                kmax = b // 128
                up = [psOP.tile([128, SC], F32, tag="op", name="up")
                      for _ in range(ND)]
                zp = psOP.tile([128, SC], F32, tag="op", name="zp")
                ekt = {}
                pend = []

                def emit_pair(t):
                    offu = max(0, 128 * 2 * t - a)
                    nc.tensor.matmul(
                        zp[:, offu:cw], ones8[:], ekt[t][:, :, offu:cw],
                        start=(t == 0), stop=(t == kmax // 2 - 1),
                        perf_mode=DR, skip_group_check=True)
                    for d0 in range(ND):
                        nc.tensor.matmul(
                            up[d0][:, offu:cw],
                            xS8t[:, t, :, d0 * 128:(d0 + 1) * 128],
                            ekt[t][:, :, offu:cw],
                            start=(t == 0), stop=(t == kmax // 2 - 1),
                            perf_mode=DR, skip_group_check=True)

                for k in range(kmax):
                    start_col = max(a, 128 * k)
                    off = start_col - a
                    w = b - start_col
                    sp = psMM.tile([128, SC], F32, tag="mm", name="mm")
                    nc.tensor.matmul(
                        sp[:, off:off + w], x8[:, :, 128 * k:128 * (k + 1)],
                        qm8[h][:, :, start_col:start_col + w],
                        start=True, stop=True, perf_mode=DR)
                    t, i = k // 2, k % 2
                    if i == 0:
                        ekt[t] = work.tile([128, 2, SC], F8, tag=f"ek{t % 2}",
                                           name=f"ek{t % 2}")
                    nc.scalar.activation(
                        out=ekt[t][:, i, off:off + w], in_=sp[:, off:off + w],
                        func=AF.Exp, scale=SCALE8)
                    if 128 * k >= a:  # diagonal band: causal mask
                        meng = nc.vector if k % 2 == 0 else nc.gpsimd
                        meng.tensor_mul(
                            out=ekt[t][:, i, off:off + 128],
                            in0=ekt[t][:, i, off:off + 128], in1=cmask8[:])
                    if i == 1:
                        offe = max(0, 128 * 2 * t - a)
                        if off > offe:
                            nc.gpsimd.memset(ekt[t][:, 1, offe:off], 0.0)
                        pend.append(t)
                        if len(pend) > 0:
                            emit_pair(pend.pop(0))
                    pump(pre_rate, post_rate)
                for t in pend:
                    emit_pair(t)
                # normalize on the psum evacuation: un = (x^T ek) * (16/Z)
                zb = work.tile([128, SC], F32, tag="zb", name="zb")
                nc.vector.reciprocal_approx_fast(out=zb[:, 0:cw],
                                                 in_=zp[:, 0:cw])
                for d0 in range(ND):
                    nc.vector.tensor_mul(out=un8[h][:, d0, a:b],
                                         in0=up[d0][:, 0:cw], in1=zb[:, 0:cw])

            # ---- main schedule: 3 query sweeps so the post pipeline of
            # tokens 0:768 overlaps the rest of attention; only 768:1024
            # drains as a tail
            sweeps = [(0, 512), (512, 1024)]
            segs = [(h, a, b) for (a, b) in sweeps for h in range(H)]
            P = [post_chunk_parts(*POST[c], tail=(c >= 2)) for c in range(4)]

            def prefetch_for(i):
                if i >= len(segs):
                    return []
                h, a, b = segs[i]
                if a == 0:
                    return seg_prefetch(h, 0)
                if a == 512:
                    return seg_prefetch(h, 1)  # covers cols 512:1024
                return []

            for th in prefetch_for(0):
                th()
            for i, (h, a, b) in enumerate(segs):
                prefq.extend(prefetch_for(i + 1))
                if i == 4:
                    postq.extend(P[0]["pre"])
                    postq.extend(P[1]["pre"])
                if i == 5:
                    while postq:  # ensure both stats are emitted
                        postq.pop(0)()
                    # ONE ACT table episode for both in-attention LN1
                    # chains; warm burst keeps the clock up across it
                    warm_pe(8)
                    P[0]["chain1"]()
                    P[1]["chain1"]()
                    postq.extend(P[0]["mid"])
                    postq.extend(P[1]["mid"])
                seg_attention(h, a, b, 1, 3)
                warm_pe(1)
                while prefq:
                    prefq.pop(0)()
            while postq:
                postq.pop(0)()
                warm_pe(4)
            # drain: c2/c3 pipelines staggered; tail stats live in "mm" and
            # tail ff1 in "op", so the only cross-chunk psC ordering left is
            # st2: emit P2.chain2 before P3's ff2 allocations
            P[0]["chain2"]()
            warm_pe(4)
            P[1]["chain2"]()
            warm_pe(4)
            for it in P[2]["pre"]:
                it()
                warm_pe(4)
            P[2]["chain1"]()
            warm_pe(4)
            m2 = list(P[2]["mid"])
            p3 = list(P[3]["pre"])
            while m2 or p3:
                if m2:
                    m2.pop(0)()
                    warm_pe(3)
                if p3:
                    p3.pop(0)()
                    warm_pe(3)
            P[3]["chain1"]()
            warm_pe(4)
            m3 = list(P[3]["mid"])
            for n, it in enumerate(m3):
                if n == NF:  # after P3's ff1 items, before its ff2 allocs
                    P[2]["chain2"]()
                    warm_pe(4)
                it()
                warm_pe(3)
            P[3]["chain2"]()

    nc.compile()
    return nc


def _np_reference(x, attention_mask, wq, wk, wv, wo_w, wo_b, ln1_g, ln1_b,
                  ff1_w, ff1_b, ff2_w, ff2_b, ln2_g, ln2_b):
    """Numpy fallback (only used if attention_mask has zeros)."""
    def ln(t, g, b):
        mu = t.mean(-1, keepdims=True)
        var = t.var(-1, keepdims=True)
        return (t - mu) / np.sqrt(var + LN_EPS) * g + b
    Bn, Sn, Dn = x.shape
    q = np.einsum('bsd,hed->bhse', x, wq)
    k = np.einsum('bsd,hed->bhse', x, wk)
    v = np.einsum('bsd,hed->bhse', x, wv)
    sc = np.einsum('bhse,bhte->bhst', q, k) / np.sqrt(np.float32(Dn))
    idx = np.arange(Sn)
    causal = idx[None, :] > idx[:, None]
    m = attention_mask.astype(bool)
    valid = m[:, None, :] & m[:, :, None]
    cond = causal[None] | ~valid
    sc = np.where(cond[:, None], -np.inf, sc)
    sc = sc - np.nanmax(np.where(np.isinf(sc), np.nan, sc), axis=-1,
                        keepdims=True)
    e = np.exp(sc)
    e = np.where(np.isnan(e), 0.0, e)
    att = e / np.maximum(e.sum(-1, keepdims=True), 1e-30)
    ho = np.einsum('bhst,bhte->bhse', att, v)
    cat = np.transpose(ho, (0, 2, 1, 3)).reshape(Bn, Sn, -1)
    mh = cat @ wo_w.T + wo_b
    x1 = ln(x + mh, ln1_g, ln1_b)
    hh = np.maximum(x1 @ ff1_w.T + ff1_b, 0.0)
    ff = hh @ ff2_w.T + ff2_b
    return ln(x1 + ff, ln2_g, ln2_b).astype(np.float32)


def _prep_inputs(inputs):
    bf = ml_dtypes.bfloat16
    f8 = ml_dtypes.float8_e4m3
    f32 = np.float32

    def dr2(wT):
        # [256, N] contraction-major -> DoubleRow [128, 2, N]
        return np.ascontiguousarray(
            wT.reshape(2, 128, -1).transpose(1, 0, 2)).astype(f8)

    def drp(wT):
        # [1024, N] -> pair-tile layout [128, 4, 2, N]
        n = wT.shape[1]
        return np.ascontiguousarray(
            wT.reshape(4, 2, 128, n).transpose(2, 0, 1, 3)).astype(f8)

    x = np.asarray(inputs["x"], f32)
    wq = np.asarray(inputs["wq"], f32)   # [H, E, D]
    wk = np.asarray(inputs["wk"], f32)
    wv = np.asarray(inputs["wv"], f32)
    woT = np.asarray(inputs["wo_w"], f32).T                    # [HE, D]
    ln1_g = np.asarray(inputs["ln1_g"], f32)
    ln1_b = np.asarray(inputs["ln1_b"], f32)
    ff1w = np.asarray(inputs["ff1_w"], f32)                    # [FF, D]
    ff1T = (ff1w * ln1_g[None, :]).T                           # gamma folded
    ff2T = np.asarray(inputs["ff2_w"], f32).T                  # [FF, D]
    ff1b = np.asarray(inputs["ff1_b"], f32) + ff1w @ ln1_b     # beta folded

    # fold the attention projections:
    #   scores_h = x^T M_h x, M_h[d1,d2] = sum_e wk[h,e,d1] wq[h,e,d2]
    #   head-out mix G_h[d,d'] = sum_e wv[h,e,d] wo_h[e,d']
    M8 = np.stack([dr2(MS * (wk[hh].T @ wq[hh]).T) for hh in range(H)],
                  axis=1)                                      # [128,H,2,D]
    G8 = np.stack([
        dr2(MS * (wv[hh].T @ woT[hh * E:(hh + 1) * E, :]))
        for hh in range(H)], axis=1)

    par = np.concatenate([
        (1024.0 * np.asarray(inputs["wo_b"], f32)).reshape(2, 128),
        (WS * ff1b).reshape(8, 128),
        (256.0 * (np.asarray(inputs["ff2_b"], f32) + ln1_b)).reshape(2, 128),
        (256.0 * ln1_g).reshape(2, 128),
        np.asarray(inputs["ln2_g"], f32).reshape(2, 128),
        np.asarray(inputs["ln2_b"], f32).reshape(2, 128),
    ], axis=0).T.copy()                                        # [128, 18]

    shared = dict(
        M8=M8, G8=G8, par=par,
        ff18=dr2(ff1T * WS), ff28=drp(ff2T * WS),
    )
    in_maps = []
    for b in range(B):
        xT = np.ascontiguousarray(x[b].T)  # [D, S]
        m = dict(shared)
        m["x8"] = dr2(xT)
        m["xS8"] = drp(x[b])               # [128, 4, 2, D] token-major
        m["x1k"] = (1024.0 * xT).astype(bf).reshape(ND, 128, S)
        in_maps.append(m)
    return in_maps


def run_sharded(inputs, trace=False, trace_kwargs=None):
    if "nc" not in _CACHE:
        _CACHE["nc"] = _build()
    nc = _CACHE["nc"]
    in_maps = _prep_inputs(inputs)
    res = run_bass_kernel_spmd(nc, in_maps, list(range(N_CORES)), trace=trace,
                               **(trace_kwargs or {}))
    outs = []
    for b in range(B):
        r = np.asarray(res.results[b]["out"], np.float32).reshape(D, S)
        outs.append(r.T)
    return np.stack(outs), res


def kernel(**inputs) -> np.ndarray:
    mask = np.asarray(inputs["attention_mask"])
    if not np.all(mask != 0):
        return _np_reference(**{k: np.asarray(v) for k, v in inputs.items()})
    out, _ = run_sharded(inputs, trace=False)
    return out


# revision 26
# speedup vs baseline: 1.0489x; 1.0489x over previous
"""Trainium2 Bass kernel for nn_DecoderBlock (B=8, S=1024, D=256, H=4 heads
of full width 256, FF=1024).

Strategy: pure data parallelism — B=8 batch elements across 8 NeuronCores,
zero collectives. Per core, one full decoder block in "transposed" activation
layout (features on SBUF partitions, tokens on the free dim).

v5: all large matmuls run fp8(e4m3) DoubleRow (2 contraction rows/cycle);
host-folded attention projections kill PSUM-evacuation traffic (only
ACT/DVE can read PSUM):

  scores_h = x^T M_h x   with M_h = wk_h wq_h^T   (no K projection)
  mh       = sum_h G_h^T (x^T exp_h / Z_h),  G_h = wv_h wo_h  (no V proj;
             the normalize rides the x^T exp psum evacuation)

Z comes from a ones(1/16) matmul so recip lands pre-scaled for fp8 u/Z.
Weight scales (x16/x64) fold into the exp scale and 1024x/256x-scaled
LN residuals (LN is scale-invariant; eps rescales). LN1 gamma/beta fold
into ff1 weights/bias and the r2 residual op.

Schedule notes (from trace analysis):
 - The PE HAM clock-gate halves the clock when the PE idles; every
   potential bubble is padded with warm matmuls into a dedicated psum
   bank (WAW-only, so a warm never waits on another engine).
 - The ACT function-table reload costs ~1.3us; any Square/Sqrt between
   attention Exps forces a reload pair. The two in-attention LN1 chains
   are emitted back-to-back as ONE episode; all other non-Exp ACT work
   (relu, LN2 apply) is deferred until after the last Exp.
 - Inputs arrive as 9 consolidated DMAs split across the sync + scalar
   HWDGE queues.

The attention_mask input is all ones per the problem spec (causal mask
only); if a mask with zeros shows up, we fall back to a numpy reference.
"""

import numpy as np
import ml_dtypes

import concourse.bass as bass
import concourse.mybir as mybir
import concourse.tile as tile
from concourse import bacc
from concourse.bass_utils import run_bass_kernel_spmd

F32 = mybir.dt.float32
BF16 = mybir.dt.bfloat16
F8 = mybir.dt.float8e4
AF = mybir.ActivationFunctionType
ALU = mybir.AluOpType
DR = mybir.MatmulPerfMode.DoubleRow

N_CORES = 8
B, S, D, H, E, HE, FF = 8, 1024, 256, 4, 256, 1024, 1024
SC = 512          # attention query chunk
NJ = S // SC
ND = D // 128
NF = FF // 128
NT = S // 128
LN_EPS = 1e-5
WS = 16.0
MS = 64.0                            # M/G host-fold scale
SCALE8 = (1.0 / 16.0) / MS           # exp reads 64x-scaled scores
EPS1 = (MS * 16.0) ** 2 * LN_EPS     # LN1 eps for 1024x-scaled r1
EPS2 = 65536.0 * LN_EPS              # LN2 eps for 256x-scaled r2
CW = 256
POST = [(0, 256), (256, 512), (512, 768), (768, 1024)]

_CACHE = {}


def _build():
    nc = bacc.Bacc("TRN2", target_bir_lowering=False, debug=False,
                   num_devices=N_CORES)

    # ---- DRAM parameters (consolidated: 9 input DMAs total) ----
    x8_d = nc.dram_tensor("x8", [128, ND, S], F8, kind="ExternalInput")
    xS8_d = nc.dram_tensor("xS8", [128, 4, 2, D], F8, kind="ExternalInput")
    x1k_d = nc.dram_tensor("x1k", [ND, 128, S], BF16, kind="ExternalInput")
    M8_d = nc.dram_tensor("M8", [128, H, ND, D], F8, kind="ExternalInput")
    G8_d = nc.dram_tensor("G8", [128, H, ND, D], F8, kind="ExternalInput")
    ff18_d = nc.dram_tensor("ff18", [128, ND, FF], F8, kind="ExternalInput")
    ff28_d = nc.dram_tensor("ff28", [128, 4, 2, D], F8, kind="ExternalInput")
    # packed per-partition params: wo_b1k(2) ff1b16(8) rb2(2) g256(2)
    # ln2_g(2) ln2_b(2)
    par_d = nc.dram_tensor("par", [128, 18], F32, kind="ExternalInput")
    out_d = nc.dram_tensor("out", [ND, 128, S], F32, kind="ExternalOutput")

    with tile.TileContext(nc) as tc:
        with tc.tile_pool(name="consts", bufs=1) as consts, \
             tc.tile_pool(name="acts", bufs=1) as acts, \
             tc.tile_pool(name="work", bufs=2) as work, \
             tc.tile_pool(name="psMM", bufs=2, space="PSUM") as psMM, \
             tc.tile_pool(name="psOP", bufs=3, space="PSUM") as psOP, \
             tc.tile_pool(name="psC", bufs=2, space="PSUM") as psC, \
             tc.tile_pool(name="psW", bufs=1, space="PSUM") as psW:

            ones_bf = consts.tile([128, 128], BF16, tag="onesb", name="onesb")
            nc.vector.memset(ones_bf[:], 1.0)
            # Z-reduction operand: 1/16 so zb = recip(Z/16) = 16/Z lands
            # pre-scaled for the fp8 range of u/Z
            ones8 = consts.tile([128, 2, 128], F8, tag="ones8", name="ones8")
            nc.gpsimd.memset(ones8[:], 1.0 / 16.0)
            invd_bf = consts.tile([128, 128], BF16, tag="invd", name="invd")
            nc.vector.memset(invd_bf[:], 1.0 / D)
            eps1_t = consts.tile([128, 1], F32, tag="eps1", name="eps1")
            nc.vector.memset(eps1_t[:], EPS1)
            eps2_t = consts.tile([128, 1], F32, tag="eps2", name="eps2")
            nc.vector.memset(eps2_t[:], EPS2)
            # multiplicative causal mask for a diagonal 128x128 block of a
            # transposed [key, query] exp tile: 1 where query >= key else 0
            cmaskf = consts.tile([128, 128], F32, tag="cmaskf", name="cmaskf")
            nc.gpsimd.memset(cmaskf[:], 1.0)
            nc.gpsimd.affine_select(
                out=cmaskf[:], in_=cmaskf[:],
                compare_op=ALU.is_ge, fill=0.0,
                base=0, pattern=[[1, 128]], channel_multiplier=-1,
            )
            cmask8 = consts.tile([128, 128], F8, tag="cmask", name="cmask")
            nc.vector.tensor_copy(out=cmask8[:], in_=cmaskf[:])

            # warm matmuls write a dedicated psum bank (WAW on itself only,
            # so a warm NEVER waits on another engine): keep the PE HAM
            # clock-gate at 8/8 by padding every potential PE bubble
            warm_rhs = consts.tile([128, SC], BF16, tag="wrhs", name="wrhs")
            nc.gpsimd.memset(warm_rhs[:], 0.0)
            wp = psW.tile([128, SC], F32, tag="w", name="wp")

            def warm_pe(n, wide=True):
                for _ in range(n):
                    if wide:
                        nc.tensor.matmul(wp[:], ones_bf[:], warm_rhs[:],
                                         start=True, stop=True,
                                         skip_group_check=True)
                    else:
                        nc.tensor.matmul(wp[:, 0:128], ones_bf[:],
                                         warm_rhs[:, 0:128],
                                         start=True, stop=True,
                                         skip_group_check=True)

            warm_pe(12)

            # ---- inputs: consolidated DMAs split across the two HWDGE
            # queues in first-use order
            def loadq(eng, dram, shape, dt, tag):
                t = consts.tile(shape, dt, tag=tag, name=tag)
                eng.dma_start(out=t[:], in_=dram[:])
                return t

            x8 = loadq(nc.sync, x8_d, [128, ND, S], F8, "x8")
            M8t = loadq(nc.sync, M8_d, [128, H, ND, D], F8, "M8")
            xS8t = loadq(nc.sync, xS8_d, [128, 4, 2, D], F8, "xS8")
            G8t = loadq(nc.scalar, G8_d, [128, H, ND, D], F8, "G8")
            par = loadq(nc.sync, par_d, [128, 18], F32, "par")
            x1k = []
            for i in range(ND):
                t = consts.tile([128, S], BF16, tag=f"x1k{i}", name=f"x1k{i}")
                nc.sync.dma_start(out=t[:], in_=x1k_d[i])
                x1k.append(t)
            ff18 = loadq(nc.scalar, ff18_d, [128, ND, FF], F8, "ff18")
            ff28t = loadq(nc.scalar, ff28_d, [128, 4, 2, D], F8, "ff28")

            wo_b1k = [par[:, 0:1], par[:, 1:2]]
            ff1b16 = [par[:, 2 + i:3 + i] for i in range(NF)]
            rb2 = [par[:, 10:11], par[:, 11:12]]
            g256 = [par[:, 12:13], par[:, 13:14]]
            ln2_g = [par[:, 14:15], par[:, 15:16]]
            ln2_b = [par[:, 16:17], par[:, 17:18]]

            # ---- persistent activation tiles ----
            qm8 = [acts.tile([128, 2, S], F8, tag=f"qm{h}", name=f"qm{h}")
                   for h in range(H)]
            un8 = [acts.tile([128, 2, S], F8, tag=f"un{h}", name=f"un{h}")
                   for h in range(H)]
            x18 = acts.tile([128, 2, S], F8, tag="x18", name="x18")
            x1z = [acts.tile([128, S], BF16, tag=f"x1z{d0}", name=f"x1z{d0}")
                   for d0 in range(ND)]
            h8 = [acts.tile([128, 2, S], F8, tag=f"h8{c}", name=f"h8{c}")
                  for c in range(4)]
            r1b = [acts.tile([128, S], BF16, tag=f"r1b{d0}", name=f"r1b{d0}")
                   for d0 in range(ND)]
            r2b = [acts.tile([128, S], BF16, tag=f"r2b{d0}", name=f"r2b{d0}")
                   for d0 in range(ND)]

            # ---- work-item pump ----
            prefq = []
            postq = []

            def pump(npre, npost):
                for _ in range(npre):
                    if prefq:
                        prefq.pop(0)()
                for _ in range(npost):
                    if postq:
                        postq.pop(0)()

            def proj_qm(h, e0, j):
                # qmT[e0 block, j cols] = (M_h^T x)[.,.]  (scale 64);
                # evacuation engine: ACT Copy is table-free, so j0 segs
                # (ACT has exp slack there) take ACT, j1 takes DVE
                cols = slice(j * SC, (j + 1) * SC)
                p = psMM.tile([128, SC], F32, tag="mm", name="mm")
                c0 = e0 * 128
                nc.tensor.matmul(p[:], M8t[:, h, :, c0:c0 + 128],
                                 x8[:, :, cols], start=True, stop=True,
                                 perf_mode=DR)
                if j == 0:
                    nc.scalar.copy(out=qm8[h][:, e0, cols], in_=p[:])
                else:
                    nc.vector.tensor_copy(out=qm8[h][:, e0, cols], in_=p[:])

            def seg_prefetch(h, j):
                return [lambda: proj_qm(h, 0, j), lambda: proj_qm(h, 1, j)]

            # ---- post-attention pipeline, per 256-token chunk ----
            def post_chunk_parts(a, b, tail):
                cw = b - a
                cols = slice(a, b)
                cell = {}
                # tail chunks run after attention: spread their psums over
                # the then-idle attention pools to decouple pipeline stages
                wo_pool, wo_tag = (psMM, "mm") if tail else (psC, "mmc")
                f1_pool, f1_tag = (psOP, "op") if tail else (psC, "mmc")

                def wo_item(d0):
                    pp = wo_pool.tile([128, SC], F32, tag=wo_tag, name="mmc")
                    for c in range(H):
                        nc.tensor.matmul(
                            pp[:, 0:cw], G8t[:, c, :, d0 * 128:(d0 + 1) * 128],
                            un8[c][:, :, cols], start=(c == 0),
                            stop=(c == H - 1), perf_mode=DR)
                    # r1b = 1024*(mh + wo_b + x), bf16 (LN scale-invariant)
                    nc.vector.scalar_tensor_tensor(
                        out=r1b[d0][:, cols], in0=pp[:, 0:cw],
                        scalar=wo_b1k[d0], in1=x1k[d0][:, cols],
                        op0=ALU.add, op1=ALU.add)

                def stats_item(src, key):
                    # mup in [:, 0:cw], m2p in [:, CW:CW+cw] of ONE psum
                    # bank: the first start=True matmul marks the whole 2KB
                    # bank pending-zero, later start=False write through.
                    st = wo_pool.tile([128, SC], F32, tag=wo_tag, name="st")
                    sq = [work.tile([128, CW], BF16, tag=f"sq{d0}",
                                    name=f"sq{d0}") for d0 in range(ND)]
                    for d0 in range(ND):
                        nc.gpsimd.tensor_mul(out=sq[d0][:, 0:cw],
                                             in0=src[d0][:, cols],
                                             in1=src[d0][:, cols])
                    for d0 in range(ND):
                        nc.tensor.matmul(st[:, 0:cw], invd_bf[:],
                                         src[d0][:, cols], start=(d0 == 0),
                                         stop=False, skip_group_check=True)
                    for d0 in range(ND):
                        nc.tensor.matmul(st[:, CW:CW + cw], invd_bf[:],
                                         sq[d0][:, 0:cw], start=False,
                                         stop=(d0 == ND - 1),
                                         skip_group_check=True)
                    cell[key] = st

                def chain_common(st, eps_t):
                    mup = st[:, 0:cw]
                    m2p = st[:, CW:CW + cw]
                    musq = work.tile([128, CW], F32, tag="musq", name="musq")
                    nc.scalar.activation(out=musq[:, 0:cw], in_=mup,
                                         func=AF.Square)
                    var = work.tile([128, CW], F32, tag="var", name="var")
                    nc.vector.tensor_sub(out=var[:, 0:cw], in0=m2p,
                                         in1=musq[:, 0:cw])
                    sd = work.tile([128, CW], F32, tag="sd", name="sd")
                    nc.scalar.activation(out=sd[:, 0:cw], in_=var[:, 0:cw],
                                         func=AF.Sqrt, bias=eps_t)
                    rstd = work.tile([128, CW], F32, tag="rstd", name="rstd")
                    nc.vector.reciprocal_approx_fast(out=rstd[:, 0:cw],
                                                     in_=sd[:, 0:cw])
                    return mup, rstd

                def chain1_item():
                    mup, rstd = chain_common(cell["st1"], eps1_t)
                    for d0 in range(ND):
                        u = work.tile([128, CW], F32, tag=f"u{d0}",
                                      name=f"u{d0}")
                        nc.vector.tensor_sub(out=u[:, 0:cw],
                                             in0=r1b[d0][:, cols], in1=mup)
                        # z (normalized, un-affine) feeds ff1 (gamma/beta
                        # folded into ff1 weights); x1z = 256*g1*z for the
                        # r2 residual (Pool does plain tensor-tensor only,
                        # so pre-scale rstd by gamma on DVE)
                        nc.vector.tensor_mul(out=x18[:, d0, cols],
                                             in0=u[:, 0:cw], in1=rstd[:, 0:cw])
                        gr = work.tile([128, CW], F32, tag=f"gr{d0}",
                                       name=f"gr{d0}")
                        nc.vector.tensor_scalar(
                            out=gr[:, 0:cw], in0=rstd[:, 0:cw],
                            scalar1=g256[d0], scalar2=None, op0=ALU.mult)
                        nc.gpsimd.tensor_mul(out=x1z[d0][:, cols],
                                             in0=u[:, 0:cw], in1=gr[:, 0:cw])

                def ff1_item(f0):
                    fp = f1_pool.tile([128, SC], F32, tag=f1_tag, name="mmc")
                    c0 = f0 * 128
                    nc.tensor.matmul(fp[:, 0:cw], ff18[:, :, c0:c0 + 128],
                                     x18[:, :, cols], start=True, stop=True,
                                     perf_mode=DR)
                    if tail and f0 % 2 == 0:
                        # ACT is exp-free after attention
                        nc.scalar.activation(
                            out=h8[f0 // 2][:, f0 % 2, cols], in_=fp[:, 0:cw],
                            func=AF.Relu, bias=ff1b16[f0])
                    else:
                        nc.vector.tensor_scalar(
                            out=h8[f0 // 2][:, f0 % 2, cols], in0=fp[:, 0:cw],
                            scalar1=ff1b16[f0], scalar2=0.0,
                            op0=ALU.add, op1=ALU.max)

                def ff2_item(d0):
                    fp = psC.tile([128, SC], F32, tag="mmc", name="mmc")
                    for c in range(4):
                        nc.tensor.matmul(
                            fp[:, 0:cw],
                            ff28t[:, c, :, d0 * 128:(d0 + 1) * 128],
                            h8[c][:, :, cols], start=(c == 0), stop=(c == 3),
                            perf_mode=DR)
                    # r2b = 256*(ff + ff2_b + x1): x1z carries 256*g1*z,
                    # rb2 carries 256*(ff2_b + b1)
                    nc.vector.scalar_tensor_tensor(
                        out=r2b[d0][:, cols], in0=fp[:, 0:cw],
                        scalar=rb2[d0], in1=x1z[d0][:, cols],
                        op0=ALU.add, op1=ALU.add)

                def chain2_item():
                    mup, rstd = chain_common(cell["st2"], eps2_t)
                    for d0 in range(ND):
                        u = work.tile([128, CW], F32, tag=f"u{d0}",
                                      name=f"u{d0}")
                        nc.vector.tensor_sub(out=u[:, 0:cw],
                                             in0=r2b[d0][:, cols], in1=mup)
                        t = work.tile([128, CW], F32, tag=f"t{d0}",
                                      name=f"t{d0}")
                        nc.vector.tensor_mul(out=t[:, 0:cw], in0=u[:, 0:cw],
                                             in1=rstd[:, 0:cw])
                        ow = work.tile([128, CW], F32, tag=f"ow{d0}",
                                       name=f"ow{d0}")
                        nc.scalar.activation(out=ow[:, 0:cw], in_=t[:, 0:cw],
                                             func=AF.Identity,
                                             scale=ln2_g[d0], bias=ln2_b[d0])
                        nc.sync.dma_start(out=out_d[d0][:, cols],
                                          in_=ow[:, 0:cw])

                return dict(
                    pre=[lambda: wo_item(0), lambda: wo_item(1),
                         lambda: stats_item(r1b, "st1")],
                    chain1=chain1_item,
                    mid=[*(lambda f0=f0: ff1_item(f0) for f0 in range(NF)),
                         lambda: ff2_item(0), lambda: ff2_item(1),
                         lambda: stats_item(r2b, "st2")],
                    chain2=chain2_item,
                )

            # ---- attention segment: head h, query window [a, b) ----
            def seg_attention(h, a, b, pre_rate, post_rate):
                cw = b - a
                kmax = b // 128
                up = [psOP.tile([128, SC], F32, tag="op", name="up")
                      for _ in range(ND)]
                zp = psOP.tile([128, SC], F32, tag="op", name="zp")
                ekt = {}
                pend = []

                def emit_pair(t):
                    offu = max(0, 128 * 2 * t - a)
                    nc.tensor.matmul(
                        zp[:, offu:cw], ones8[:], ekt[t][:, :, offu:cw],
                        start=(t == 0), stop=(t == kmax // 2 - 1),
                        perf_mode=DR, skip_group_check=True)
                    for d0 in range(ND):
                        nc.tensor.matmul(
                            up[d0][:, offu:cw],
                            xS8t[:, t, :, d0 * 128:(d0 + 1) * 128],
                            ekt[t][:, :, offu:cw],
                            start=(t == 0), stop=(t == kmax // 2 - 1),
                            perf_mode=DR, skip_group_check=True)

                for k in range(kmax):
                    start_col = max(a, 128 * k)
                    off = start_col - a
                    w = b - start_col
                    sp = psMM.tile([128, SC], F32, tag="mm", name="mm")
                    nc.tensor.matmul(
                        sp[:, off:off + w], x8[:, :, 128 * k:128 * (k + 1)],
                        qm8[h][:, :, start_col:start_col + w],
                        start=True, stop=True, perf_mode=DR)
                    t, i = k // 2, k % 2
                    if i == 0:
                        ekt[t] = work.tile([128, 2, SC], F8, tag=f"ek{t % 2}",
                                           name=f"ek{t % 2}")
                    nc.scalar.activation(
                        out=ekt[t][:, i, off:off + w], in_=sp[:, off:off + w],
                        func=AF.Exp, scale=SCALE8)
                    if 128 * k >= a:  # diagonal band: causal mask
                        meng = nc.vector if k % 2 == 0 else nc.gpsimd
                        meng.tensor_mul(
                            out=ekt[t][:, i, off:off + 128],
                            in0=ekt[t][:, i, off:off + 128], in1=cmask8[:])
                    if i == 1:
                        offe = max(0, 128 * 2 * t - a)
                        if off > offe:
                            nc.gpsimd.memset(ekt[t][:, 1, offe:off], 0.0)
                        pend.append(t)
                        if len(pend) > 1:
                            emit_pair(pend.pop(0))
                    pump(pre_rate, post_rate)
                for t in pend:
                    emit_pair(t)
                # normalize on the psum evacuation: un = (x^T ek) * (16/Z)
                zb = work.tile([128, SC], F32, tag="zb", name="zb")
                nc.vector.reciprocal_approx_fast(out=zb[:, 0:cw],
                                                 in_=zp[:, 0:cw])
                for d0 in range(ND):
                    nc.vector.tensor_mul(out=un8[h][:, d0, a:b],
                                         in0=up[d0][:, 0:cw], in1=zb[:, 0:cw])

            # ---- main schedule: 3 query sweeps so the post pipeline of
            # tokens 0:768 overlaps the rest of attention; only 768:1024
            # drains as a tail
            sweeps = [(0, 512), (512, 1024)]
            segs = [(h, a, b) for (a, b) in sweeps for h in range(H)]
            P = [post_chunk_parts(*POST[c], tail=(c >= 2)) for c in range(4)]

            def prefetch_for(i):
                if i >= len(segs):
                    return []
                h, a, b = segs[i]
                if a == 0:
                    return seg_prefetch(h, 0)
                if a == 512:
                    return seg_prefetch(h, 1)  # covers cols 512:1024
                return []

            for th in prefetch_for(0):
                th()
            for i, (h, a, b) in enumerate(segs):
                prefq.extend(prefetch_for(i + 1))
                if i == 4:
                    postq.extend(P[0]["pre"])
                    postq.extend(P[1]["pre"])
                if i == 5:
                    while postq:  # ensure both stats are emitted
                        postq.pop(0)()
                    # ONE ACT table episode for both in-attention LN1
                    # chains; warm burst keeps the clock up across it
                    warm_pe(8)
                    P[0]["chain1"]()
                    P[1]["chain1"]()
                    postq.extend(P[0]["mid"])
                    postq.extend(P[1]["mid"])
                seg_attention(h, a, b, 1, 2)
                warm_pe(1)
                while prefq:
                    prefq.pop(0)()
            while postq:
                postq.pop(0)()
                warm_pe(4)
            # drain: c2/c3 pipelines staggered; tail stats live in "mm" and
            # tail ff1 in "op", so the only cross-chunk psC ordering left is
            # st2: emit P2.chain2 before P3's ff2 allocations
            P[0]["chain2"]()
            warm_pe(4)
            P[1]["chain2"]()
            warm_pe(4)
            for it in P[2]["pre"]:
                it()
                warm_pe(4)
            P[2]["chain1"]()
            warm_pe(4)
            m2 = list(P[2]["mid"])
            p3 = list(P[3]["pre"])
            while m2 or p3:
                if m2:
                    m2.pop(0)()
                    warm_pe(3)
                if p3:
                    p3.pop(0)()
                    warm_pe(3)
            P[3]["chain1"]()
            warm_pe(4)
            m3 = list(P[3]["mid"])
            for n, it in enumerate(m3):
                if n == NF:  # after P3's ff1 items, before its ff2 allocs
                    P[2]["chain2"]()
                    warm_pe(4)
                it()
                warm_pe(3)
            P[3]["chain2"]()

    nc.compile()
    return nc


def _np_reference(x, attention_mask, wq, wk, wv, wo_w, wo_b, ln1_g, ln1_b,
                  ff1_w, ff1_b, ff2_w, ff2_b, ln2_g, ln2_b):
    """Numpy fallback (only used if attention_mask has zeros)."""
    def ln(t, g, b):
        mu = t.mean(-1, keepdims=True)
        var = t.var(-1, keepdims=True)
        return (t - mu) / np.sqrt(var + LN_EPS) * g + b
    Bn, Sn, Dn = x.shape
    q = np.einsum('bsd,hed->bhse', x, wq)
    k = np.einsum('bsd,hed->bhse', x, wk)
    v = np.einsum('bsd,hed->bhse', x, wv)
    sc = np.einsum('bhse,bhte->bhst', q, k) / np.sqrt(np.float32(Dn))
    idx = np.arange(Sn)
    causal = idx[None, :] > idx[:, None]
    m = attention_mask.astype(bool)
    valid = m[:, None, :] & m[:, :, None]
    cond = causal[None] | ~valid
    sc = np.where(cond[:, None], -np.inf, sc)
    sc = sc - np.nanmax(np.where(np.isinf(sc), np.nan, sc), axis=-1,
                        keepdims=True)
    e = np.exp(sc)
    e = np.where(np.isnan(e), 0.0, e)
    att = e / np.maximum(e.sum(-1, keepdims=True), 1e-30)
    ho = np.einsum('bhst,bhte->bhse', att, v)
    cat = np.transpose(ho, (0, 2, 1, 3)).reshape(Bn, Sn, -1)
    mh = cat @ wo_w.T + wo_b
    x1 = ln(x + mh, ln1_g, ln1_b)
    hh = np.maximum(x1 @ ff1_w.T + ff1_b, 0.0)
    ff = hh @ ff2_w.T + ff2_b
    return ln(x1 + ff, ln2_g, ln2_b).astype(np.float32)


def _prep_inputs(inputs):
    bf = ml_dtypes.bfloat16
    f8 = ml_dtypes.float8_e4m3
    f32 = np.float32

    def dr2(wT):
        # [256, N] contraction-major -> DoubleRow [128, 2, N]
        return np.ascontiguousarray(
            wT.reshape(2, 128, -1).transpose(1, 0, 2)).astype(f8)

    def drp(wT):
        # [1024, N] -> pair-tile layout [128, 4, 2, N]
        n = wT.shape[1]
        return np.ascontiguousarray(
            wT.reshape(4, 2, 128, n).transpose(2, 0, 1, 3)).astype(f8)

    x = np.asarray(inputs["x"], f32)
    wq = np.asarray(inputs["wq"], f32)   # [H, E, D]
    wk = np.asarray(inputs["wk"], f32)
    wv = np.asarray(inputs["wv"], f32)
    woT = np.asarray(inputs["wo_w"], f32).T                    # [HE, D]
    ln1_g = np.asarray(inputs["ln1_g"], f32)
    ln1_b = np.asarray(inputs["ln1_b"], f32)
    ff1w = np.asarray(inputs["ff1_w"], f32)                    # [FF, D]
    ff1T = (ff1w * ln1_g[None, :]).T                           # gamma folded
    ff2T = np.asarray(inputs["ff2_w"], f32).T                  # [FF, D]
    ff1b = np.asarray(inputs["ff1_b"], f32) + ff1w @ ln1_b     # beta folded

    # fold the attention projections:
    #   scores_h = x^T M_h x, M_h[d1,d2] = sum_e wk[h,e,d1] wq[h,e,d2]
    #   head-out mix G_h[d,d'] = sum_e wv[h,e,d] wo_h[e,d']
    M8 = np.stack([dr2(MS * (wk[hh].T @ wq[hh]).T) for hh in range(H)],
                  axis=1)                                      # [128,H,2,D]
    G8 = np.stack([
        dr2(MS * (wv[hh].T @ woT[hh * E:(hh + 1) * E, :]))
        for hh in range(H)], axis=1)

    par = np.concatenate([
        (1024.0 * np.asarray(inputs["wo_b"], f32)).reshape(2, 128),
        (WS * ff1b).reshape(8, 128),
        (256.0 * (np.asarray(inputs["ff2_b"], f32) + ln1_b)).reshape(2, 128),
        (256.0 * ln1_g).reshape(2, 128),
        np.asarray(inputs["ln2_g"], f32).reshape(2, 128),
        np.asarray(inputs["ln2_b"], f32).reshape(2, 128),
    ], axis=0).T.copy()                                        # [128, 18]

    shared = dict(
        M8=M8, G8=G8, par=par,
        ff18=dr2(ff1T * WS), ff28=drp(ff2T * WS),
    )
    in_maps = []
    for b in range(B):
        xT = np.ascontiguousarray(x[b].T)  # [D, S]
        m = dict(shared)
        m["x8"] = dr2(xT)
        m["xS8"] = drp(x[b])               # [128, 4, 2, D] token-major
        m["x1k"] = (1024.0 * xT).astype(bf).reshape(ND, 128, S)
        in_maps.append(m)
    return in_maps


def run_sharded(inputs, trace=False, trace_kwargs=None):
    if "nc" not in _CACHE:
        _CACHE["nc"] = _build()
    nc = _CACHE["nc"]
    in_maps = _prep_inputs(inputs)
    res = run_bass_kernel_spmd(nc, in_maps, list(range(N_CORES)), trace=trace,
                               **(trace_kwargs or {}))
    outs = []
    for b in range(B):
        r = np.asarray(res.results[b]["out"], np.float32).reshape(D, S)
        outs.append(r.T)
    return np.stack(outs), res


def kernel(**inputs) -> np.ndarray:
    mask = np.asarray(inputs["attention_mask"])
    if not np.all(mask != 0):
        return _np_reference(**{k: np.asarray(v) for k, v in inputs.items()})
    out, _ = run_sharded(inputs, trace=False)
    return out
